# Initial kernel scaffold
#
"""Causal self-attention (B=4, T=2048, C=1024, H=16) on 8 Trainium2 cores.

Sharding: data-parallel over batch (4 groups) x tensor-parallel over heads
(2-way). Core c = 2*b + t handles batch b, heads [t*8, t*8+8).

Per-core device kernel (all matmuls fp16 in / fp32 psum accumulate):
  - qkv projection computed in transposed layout: qk^T[feat, T] so that the
    per-head Q^T/K^T tiles feed the S^T matmul directly; V computed in
    natural [T, feat] layout (it is the stationary operand of the AV matmul)
    with a ones-column appended so the AV matmul also accumulates the
    softmax denominator l[q] for free.
  - S^T[k, q] = K^T.T @ Q^T per (head, 128-wide k-block, 512-wide q-chunk),
    causal blocks only; exp via ACT (scale=1/8 folded in); additive -1e30
    mask on the 4 diagonal blocks of each q-chunk.
  - O^T_aug[65, q] = V_aug.T @ P^T accumulated over k-blocks; row 64 is l.
  - normalize via DVE reciprocal + K=1 ones-matmul partition-broadcast.
  - output projection row-parallel: out^T = W_proj_half.T @ y^T (+ b_proj on
    the t=0 core); host sums the two partials per batch and transposes.
"""

import sys

import numpy as np

from concourse import mybir, tile, bacc
from concourse import bass_utils
from concourse.bass_utils import run_bass_kernel_spmd


def _ensure_trace_support():
    """Make trace=True / BASS_TRACE runs survive on images whose antenv lacks
    axon_hooks and where artifact upload has no credentials. Both shims are
    no-ops on the untraced path."""
    try:
        import antenv.axon_hooks  # noqa: F401
    except ImportError:
        import contextlib
        import ctypes
        import os
        import types

        mod = types.ModuleType("antenv.axon_hooks")
        state = {"hook": None, "tried": False}

        def set_axon_ntff_profile_hook(hook):
            state["hook"] = hook

        def _via_ctypes(so_path):
            lib = ctypes.CDLL(so_path)
            if not hasattr(lib, "axon_start_nrt_profile"):
                return None
            lib.axon_start_nrt_profile.argtypes = [
                ctypes.POINTER(ctypes.c_int64),
                ctypes.c_size_t,
            ]
            lib.axon_start_nrt_profile.restype = ctypes.c_int64
            lib.axon_stop_nrt_profile.argtypes = [ctypes.c_char_p]
            lib.axon_stop_nrt_profile.restype = ctypes.c_int64

            @contextlib.contextmanager
            def _hook(output_dir, device_ids):
                import jax

                jax.devices()
                if device_ids:
                    ids = (ctypes.c_int64 * len(device_ids))(*device_ids)
                    rc = lib.axon_start_nrt_profile(ids, len(device_ids))
                else:
                    rc = lib.axon_start_nrt_profile(None, 0)
                if rc != 0:
                    raise RuntimeError(f"axon_start_nrt_profile rc={rc}")
                try:
                    yield
                finally:
                    lib.axon_stop_nrt_profile(str(output_dir).encode())

            return _hook

        def get_axon_ntff_profile_hook():
            if state["hook"] is None and not state["tried"]:
                state["tried"] = True
                so = os.environ.get("AXON_PJRT_SO", "/opt/axon/libaxon_pjrt.so")
                if os.path.exists(so):
                    try:
                        state["hook"] = _via_ctypes(so)
                    except OSError:
                        pass
            return state["hook"]

        mod.set_axon_ntff_profile_hook = set_axon_ntff_profile_hook
        mod.get_axon_ntff_profile_hook = get_axon_ntff_profile_hook
        sys.modules["antenv.axon_hooks"] = mod

    orig_upload = bass_utils.upload_artifacts
    if not getattr(orig_upload, "_safe_wrapped", False):
        def _safe_upload(tmpdir):
            try:
                return orig_upload(tmpdir)
            except Exception:
                return "local://" + str(tmpdir)

        _safe_upload._safe_wrapped = True
        bass_utils.upload_artifacts = _safe_upload


_ensure_trace_support()

F16 = mybir.dt.float16
F32 = mybir.dt.float32
EXPF = mybir.ActivationFunctionType.Exp
IDF = mybir.ActivationFunctionType.Identity

B, T, C, H, D = 4, 2048, 1024, 16, 64
HPC = 8          # heads per core
QC = 512         # q-chunk width
NT = T // 128    # 16 T-tiles of 128
NQC = T // QC    # 4 q-chunks
NKC = C // 128   # 8 contraction tiles for the input projections
NKP = (HPC * D) // 128  # 4 contraction tiles for the output projection
NEG = -1.0e30

_CACHE = {}


def _build():
    nc = bacc.Bacc("TRN2", target_bir_lowering=False, debug=False)

    xT = nc.dram_tensor("xT", [C, T], F16, kind="ExternalInput")
    wqk = nc.dram_tensor("wqk", [C, HPC * 128], F16, kind="ExternalInput")
    bqk = nc.dram_tensor("bqk", [HPC * 128], F32, kind="ExternalInput")
    wv = nc.dram_tensor("wv", [C, HPC * D], F16, kind="ExternalInput")
    bv = nc.dram_tensor("bv", [HPC * D], F16, kind="ExternalInput")
    wp = nc.dram_tensor("wp", [HPC * D, C], F16, kind="ExternalInput")
    bp = nc.dram_tensor("bp", [C], F32, kind="ExternalInput")
    mask = nc.dram_tensor("mask", [4, 128, QC], F32, kind="ExternalInput")
    outT = nc.dram_tensor("outT", [C, T], F32, kind="ExternalOutput")

    with tile.TileContext(nc) as tc:
        with (
            tc.tile_pool(name="wts", bufs=1) as wpool,
            tc.tile_pool(name="qk", bufs=1) as qkpool,
            tc.tile_pool(name="vy", bufs=1) as vypool,
            tc.tile_pool(name="xc", bufs=2) as xpool,
            tc.tile_pool(name="pt", bufs=4) as ppool,
            tc.tile_pool(name="tmp", bufs=4) as tpool,
            tc.tile_pool(name="st", bufs=4) as spool,
            tc.tile_pool(name="sm", bufs=4) as smallpool,
            tc.tile_pool(name="ot", bufs=2) as otpool,
            tc.tile_pool(name="dr", bufs=2, space="DRAM") as drpool,
            tc.tile_pool(name="blk", bufs=5, space="PSUM") as pspool,
            tc.tile_pool(name="av", bufs=3, space="PSUM") as opool,
        ):
            # ---- weights / constants ----
            # DMA order matters at kernel start: the first x chunk and wv
            # come first so the V projection can start ~5us in; wqk streams
            # in 8 per-m-tile slices consumed in order; everything needed
            # later (mask, wp, biases) loads behind them.
            wv_sb = wpool.tile([128, NKC, HPC * D], F16, tag="wv")
            nc.sync.dma_start(wv_sb[:], wv.ap().rearrange("(a p) m -> p a m", p=128))
            bv_sb = wpool.tile([1, HPC * D], F16, tag="bv")
            nc.sync.dma_start(bv_sb[:], bv.ap().rearrange("(o n) -> o n", o=1))
            ones16 = wpool.tile([1, 128], F16, tag="ones")
            nc.vector.memset(ones16[:], 1.0)
            # (wqk / bqk / mask / wp / bp DMAs are emitted inside chunk 0,
            # behind the first x-chunk DMA, so the V projection starts as
            # early as possible)

            # ---- persistent activation tiles ----
            q_t = [qkpool.tile([64, T], F16, tag=f"q{h}", name=f"q{h}") for h in range(HPC)]
            k_t = [qkpool.tile([64, T], F16, tag=f"k{h}", name=f"k{h}") for h in range(HPC)]
            # V with ones column: [128 part, T-tile, head, 64+1]
            v_sb = vypool.tile([128, NT, HPC, D + 1], F16, tag="v")
            nc.vector.memset(v_sb[:, :, :, D : D + 1], 1.0)
            y_t = [vypool.tile([128, T], F16, tag=f"y{g}", name=f"y{g}") for g in range(NKP)]

            # Softmax normalization runs entirely off the PE: per head the
            # denominator row l goes to a DRAM scratch and O is copied to
            # SBUF f16 (freeing the PSUM bank). One chunk's 8 l-rows are
            # then reloaded as [128, 32] so a single DVE reciprocal (cost
            # is proportional to the free dim) covers the whole chunk; each
            # head's 1/l row is partition-broadcast back via DMA and y is a
            # cheap f16 x f16 DVE multiply. The flush is deferred past the
            # next chunk's input projections so every dependency is long
            # resolved (a >3.4us PE gap would re-throttle it to 1.2 GHz).
            def emit_flush(qc, ots, l_dram):
                q0 = qc * QC
                lall = smallpool.tile([128, 32], F32, tag="lall", name="lall")
                nc.sync.dma_start(
                    lall[:],
                    l_dram[:]
                    .rearrange("a (p n) -> (a p) n", n=32)
                    .rearrange("(a p) n -> p a n", p=128),
                )
                rall = smallpool.tile([128, 32], F32, tag="rall", name="rall")
                nc.vector.reciprocal(rall[:], lall[:])
                r16 = smallpool.tile([128, 32], F16, tag="r16", name="r16")
                nc.vector.tensor_copy(r16[:], rall[:])
                r16_dram = drpool.tile([HPC, QC], F16, tag="rdram", name="rdram")
                nc.sync.dma_start(
                    r16_dram[:]
                    .rearrange("a (p n) -> (a p) n", n=32)
                    .rearrange("(a p) n -> p a n", p=128),
                    r16[:],
                )
                for h, ot in ots:
                    bcs = tpool.tile([64, QC], F16, tag="bcs", name="bcs")
                    nc.sync.dma_start(
                        bcs[:], r16_dram[h : h + 1, :].partition_broadcast(64)
                    )
                    r0 = (h % 2) * 64
                    nc.vector.tensor_mul(
                        y_t[h // 2][r0 : r0 + 64, q0 : q0 + QC], ot[:], bcs[:]
                    )

            def emit_proj(qc):
                q0 = qc * QC
                for m in range(C // 128):
                    pps = pspool.tile([128, QC], F32, tag="blk", name="pps")
                    for kk in range(NKP):
                        nc.tensor.matmul(
                            pps[:],
                            wp_sb[:, kk, m * 128 : (m + 1) * 128],
                            y_t[kk][:, q0 : q0 + QC],
                            start=(kk == 0),
                            stop=(kk == NKP - 1),
                        )
                    st = spool.tile([128, QC], F32, tag="st", name="st")
                    nc.scalar.activation(st[:], pps[:], IDF, bias=bp_sb[:, m : m + 1])
                    # stores go on the SWDGE (gpsimd) queue so they never
                    # delay latency-critical loads/broadcasts on the HWDGE
                    nc.gpsimd.dma_start(
                        outT.ap()[m * 128 : (m + 1) * 128, q0 : q0 + QC], st[:]
                    )

            pending = None  # (qc, [(h, ot_sb)...], l_dram) awaiting flush

            for qc in range(NQC):
                q0 = qc * QC
                # ---- input projections for this T-chunk ----
                xc = xpool.tile([128, NKC, QC], F16, tag="xc")
                nc.sync.dma_start(
                    xc[:],
                    xT.ap()[:, q0 : q0 + QC].rearrange("(a p) n -> p a n", p=128),
                )
                # V projection first: at qc=0 it only needs xc + wv, which
                # are the first two DMAs issued.
                for tt in range(4):
                    tg = qc * 4 + tt
                    ps = pspool.tile([128, QC], F32, tag="blk")
                    for kk in range(NKC):
                        nc.tensor.matmul(
                            ps[:],
                            xc[:, kk, tt * 128 : (tt + 1) * 128],
                            wv_sb[:, kk, :],
                            start=(kk == 0),
                            stop=False,
                        )
                    nc.tensor.matmul(
                        ps[:], ones16[0:1, 0:128], bv_sb[:], start=False, stop=True
                    )
                    nc.vector.tensor_copy(
                        v_sb[:, tg, :, 0:D],
                        ps[:].rearrange("p (h d) -> p h d", d=D),
                    )
                if qc == 0:
                    # remaining weights stream behind xc0/wv on the DMA queue
                    wqk_sb = wpool.tile([128, NKC, HPC * 128], F16, tag="wqk")
                    nc.sync.dma_start(
                        wqk_sb[:], wqk.ap().rearrange("(a p) m -> p a m", p=128)
                    )
                    bqk_sb = wpool.tile([128, HPC], F32, tag="bqk")
                    nc.sync.dma_start(
                        bqk_sb[:], bqk.ap().rearrange("(m p) -> p m", p=128)
                    )
                    mask_sb = wpool.tile([128, 4, QC], F32, tag="mask")
                    nc.sync.dma_start(mask_sb[:], mask.ap().rearrange("o p n -> p o n"))
                    wp_sb = wpool.tile([128, NKP, C], F16, tag="wp")
                    nc.sync.dma_start(
                        wp_sb[:], wp.ap().rearrange("(a p) m -> p a m", p=128)
                    )
                    bp_sb = wpool.tile([128, C // 128], F32, tag="bp")
                    nc.sync.dma_start(bp_sb[:], bp.ap().rearrange("(m p) -> p m", p=128))
                for m in range(HPC):
                    ps = pspool.tile([128, QC], F32, tag="blk")
                    for kk in range(NKC):
                        nc.tensor.matmul(
                            ps[:],
                            wqk_sb[:, kk, m * 128 : (m + 1) * 128],
                            xc[:, kk, :],
                            start=(kk == 0),
                            stop=(kk == NKC - 1),
                        )
                    nc.vector.tensor_scalar_add(
                        q_t[m][:, q0 : q0 + QC], ps[0:64, :], bqk_sb[0:64, m : m + 1]
                    )
                    nc.vector.tensor_scalar_add(
                        k_t[m][:, q0 : q0 + QC],
                        ps[64:128, :],
                        bqk_sb[64:128, m : m + 1],
                    )

                # Flush the previous chunk's normalization now (the input
                # projections above gave the PE ~25us of cover), then emit
                # the previous chunk's output projection. proj(2) is held
                # back until after attention(3) so it covers the tail
                # flush(3) chain on the PE.
                if pending is not None:
                    emit_flush(*pending)
                    if qc - 1 < NQC - 2:
                        emit_proj(qc - 1)
                pending = None

                # ---- attention for q-chunk qc, all heads ----
                n_k = qc * 4 + 4
                l_dram = drpool.tile([HPC, QC], F32, tag="ldram", name="ldram")
                ots = []
                for h in range(HPC):
                    ops = opool.tile([128, QC], F32, tag="av", name="ops")
                    for j in range(n_k):
                        o = j - (n_k - 4)  # diagonal offset, <0 for full blocks
                        c0 = 128 * o if o > 0 else 0  # cols < c0 are fully masked
                        w = QC - c0
                        sps = pspool.tile([128, QC], F32, tag="blk", name="sps")
                        nc.tensor.matmul(
                            sps[:, 0:w],
                            k_t[h][:, j * 128 : (j + 1) * 128],
                            q_t[h][:, q0 + c0 : q0 + QC],
                            start=True,
                            stop=True,
                        )
                        pt = ppool.tile([128, QC], F16, tag="pt", name="pt")
                        if o >= 0:
                            mt = tpool.tile([128, QC], F32, tag="mt", name="mt")
                            nc.vector.tensor_add(
                                mt[:, 0:w], sps[:, 0:w], mask_sb[:, o, c0:QC]
                            )
                            nc.scalar.activation(
                                pt[:, 0:w], mt[:, 0:w], EXPF, scale=0.125
                            )
                        else:
                            nc.scalar.activation(pt[:, 0:w], sps[:, 0:w], EXPF, scale=0.125)
                        nc.tensor.matmul(
                            ops[0:65, c0:QC],
                            v_sb[:, j, h, :],
                            pt[:, 0:w],
                            start=(j == 0),
                            stop=(j == n_k - 1),
                        )
                    # denominator row to DRAM scratch (DMA cannot read PSUM,
                    # so hop through SBUF); O to SBUF f16 (frees the PSUM
                    # bank for the next head)
                    lrow = smallpool.tile([1, QC], F32, tag="lrow", name="lrow")
                    nc.vector.tensor_copy(lrow[:], ops[64:65, :])
                    nc.sync.dma_start(l_dram[h : h + 1, :], lrow[:])
                    ot = otpool.tile([64, QC], F16, tag=f"ot{h}", name=f"ot{h}")
                    nc.vector.tensor_copy(ot[:], ops[0:64, :])
                    ots.append((h, ot))
                pending = (qc, ots, l_dram)

            # tail: the held-back proj(2) gives the PE ~13us of work while
            # the last chunk's flush chain (DMA/DVE) resolves, then proj(3)
            emit_proj(NQC - 2)
            emit_flush(*pending)
            emit_proj(NQC - 1)

    nc.compile()
    return nc


def _shards(W_attn, b_attn, W_proj, b_proj):
    """Per-TP-half weight shards (t = 0, 1), packed for the device layout."""
    shards = []
    for t in range(2):
        heads = range(t * HPC, (t + 1) * HPC)
        wqk = np.empty((C, HPC * 128), np.float16)
        bqk = np.empty(HPC * 128, np.float32)
        wv = np.empty((C, HPC * D), np.float16)
        bvv = np.empty(HPC * D, np.float16)
        for j, h in enumerate(heads):
            wqk[:, j * 128 : j * 128 + 64] = W_attn[:, h * D : (h + 1) * D]
            wqk[:, j * 128 + 64 : j * 128 + 128] = W_attn[:, C + h * D : C + (h + 1) * D]
            bqk[j * 128 : j * 128 + 64] = b_attn[h * D : (h + 1) * D]
            bqk[j * 128 + 64 : j * 128 + 128] = b_attn[C + h * D : C + (h + 1) * D]
            wv[:, j * D : (j + 1) * D] = W_attn[:, 2 * C + h * D : 2 * C + (h + 1) * D]
            bvv[j * D : (j + 1) * D] = b_attn[2 * C + h * D : 2 * C + (h + 1) * D]
        wp = W_proj[t * HPC * D : (t + 1) * HPC * D, :].astype(np.float16)
        bpp = (b_proj if t == 0 else np.zeros_like(b_proj)).astype(np.float32)
        shards.append((wqk, bqk, wv, bvv, np.ascontiguousarray(wp), bpp))
    return shards


def _mask_np():
    kr = np.arange(128)[:, None]
    qr = np.arange(QC)[None, :]
    m = np.empty((4, 128, QC), np.float32)
    for o in range(4):
        m[o] = np.where(kr + o * 128 <= qr, 0.0, NEG)
    return m


def _in_maps(x, W_attn, b_attn, W_proj, b_proj):
    shards = _shards(W_attn, b_attn, W_proj, b_proj)
    mask = _mask_np()
    in_maps = []
    for b in range(B):
        xTb = np.ascontiguousarray(x[b].T.astype(np.float16))
        for t in range(2):
            wqk, bqk, wv, bvv, wp, bpp = shards[t]
            in_maps.append(
                {
                    "xT": xTb,
                    "wqk": wqk,
                    "bqk": bqk,
                    "wv": wv,
                    "bv": bvv,
                    "wp": wp,
                    "bp": bpp,
                    "mask": mask,
                }
            )
    return in_maps


def _gather(results):
    out = np.empty((B, T, C), np.float32)
    for b in range(B):
        acc = results[2 * b]["outT"] + results[2 * b + 1]["outT"]
        out[b] = acc.T
    return out


def kernel(x, W_attn, b_attn, W_proj, b_proj):
    x = np.asarray(x, np.float32)
    W_attn = np.asarray(W_attn, np.float32)
    b_attn = np.asarray(b_attn, np.float32)
    W_proj = np.asarray(W_proj, np.float32)
    b_proj = np.asarray(b_proj, np.float32)

    if "nc" not in _CACHE:
        _CACHE["nc"] = _build()
    nc = _CACHE["nc"]

    in_maps = _in_maps(x, W_attn, b_attn, W_proj, b_proj)
    res = run_bass_kernel_spmd(nc, in_maps, core_ids=list(range(8)))
    return _gather(res.results)



# revision 22
# speedup vs baseline: 1.3943x; 1.3943x over previous
"""Causal self-attention (B=4, T=2048, C=1024, H=16) on 8 Trainium2 cores.

Sharding: data-parallel over batch (4 groups) x tensor-parallel over heads
(2-way). Core c = 2*b + t handles batch b, heads [t*8, t*8+8).

v2 design notes (PE-saturation schedule):
  The PE clock is governed by a DVFS controller: sustained ~100% utilization
  holds ~2.37 GHz; utilization gaps in its ~3.4us evaluation window drop it
  to ~1.2 GHz with a ~3us ramp back. The v1 kernel lost ~45-65% of its span
  to that downshift because the S->exp->AV chain stalled the PE behind the
  scalar engine. v2 restructures so the PE queue never waits:

  - attention emitted in units (pairs of 128-row k-blocks; diagonal blocks
    single) with a 2-unit lag between each unit's S matmuls and its AV
    matmuls, so every exp has ~2us of PE cover.
  - exps fused across 2 PSUM banks ([128,1024] per ACT op) to amortize the
    ~190ns ACT fixed overhead; diagonal blocks keep partial widths.
  - causal masking is multiplicative AFTER exp (f16 DVE mul against a
    [128,128] triangle tile, only on the mixed 128-col region) instead of
    additive -1e30 before it: halves DVE cost and keeps exp input PSUM-fused.
  - input projections of chunk qc+1 and the output projections are emitted
    as m-tile fillers INSIDE attention so proj work absorbs any PE slack.
  - softmax normalize without DRAM round-trips: DVE reciprocal of the l row
    (PSUM partition 64), gpsimd partition_broadcast, then one DVE mul from
    PSUM O straight into y^T f16.
  - V bias via DVE add against a broadcast tile (frees the PE ones-matmul).

Per-core math (all matmuls fp16 in / fp32 psum accumulate), as in v1:
  qkv in transposed layout q^T/k^T[feat,T]; V in [T,feat] with a ones column
  so AV also accumulates the softmax denominator; S^T = K^T.T @ Q^T per
  (head, k-block, 512-col q-chunk), causal blocks only; out^T row-parallel =
  W_proj_half.T @ y^T (+ b_proj on the t=0 core); host sums TP partials.
"""

import sys

import numpy as np

from concourse import mybir, tile, bacc
from concourse import bass_utils
from concourse.bass_utils import run_bass_kernel_spmd


def _ensure_trace_support():
    """Make trace=True / BASS_TRACE runs survive on images whose antenv lacks
    axon_hooks and where artifact upload has no credentials. Both shims are
    no-ops on the untraced path."""
    try:
        import antenv.axon_hooks  # noqa: F401
    except ImportError:
        import contextlib
        import ctypes
        import os
        import types

        mod = types.ModuleType("antenv.axon_hooks")
        state = {"hook": None, "tried": False}

        def set_axon_ntff_profile_hook(hook):
            state["hook"] = hook

        def _via_ctypes(so_path):
            lib = ctypes.CDLL(so_path)
            if not hasattr(lib, "axon_start_nrt_profile"):
                return None
            lib.axon_start_nrt_profile.argtypes = [
                ctypes.POINTER(ctypes.c_int64),
                ctypes.c_size_t,
            ]
            lib.axon_start_nrt_profile.restype = ctypes.c_int64
            lib.axon_stop_nrt_profile.argtypes = [ctypes.c_char_p]
            lib.axon_stop_nrt_profile.restype = ctypes.c_int64

            @contextlib.contextmanager
            def _hook(output_dir, device_ids):
                import jax

                jax.devices()
                if device_ids:
                    ids = (ctypes.c_int64 * len(device_ids))(*device_ids)
                    rc = lib.axon_start_nrt_profile(ids, len(device_ids))
                else:
                    rc = lib.axon_start_nrt_profile(None, 0)
                if rc != 0:
                    raise RuntimeError(f"axon_start_nrt_profile rc={rc}")
                try:
                    yield
                finally:
                    lib.axon_stop_nrt_profile(str(output_dir).encode())

            return _hook

        def get_axon_ntff_profile_hook():
            if state["hook"] is None and not state["tried"]:
                state["tried"] = True
                so = os.environ.get("AXON_PJRT_SO", "/opt/axon/libaxon_pjrt.so")
                if os.path.exists(so):
                    try:
                        state["hook"] = _via_ctypes(so)
                    except OSError:
                        pass
            return state["hook"]

        mod.set_axon_ntff_profile_hook = set_axon_ntff_profile_hook
        mod.get_axon_ntff_profile_hook = get_axon_ntff_profile_hook
        sys.modules["antenv.axon_hooks"] = mod

    orig_upload = bass_utils.upload_artifacts
    if not getattr(orig_upload, "_safe_wrapped", False):
        def _safe_upload(tmpdir):
            try:
                return orig_upload(tmpdir)
            except Exception:
                return "local://" + str(tmpdir)

        _safe_upload._safe_wrapped = True
        bass_utils.upload_artifacts = _safe_upload


_ensure_trace_support()

F16 = mybir.dt.float16
F32 = mybir.dt.float32
EXPF = mybir.ActivationFunctionType.Exp
IDF = mybir.ActivationFunctionType.Identity

B, T, C, H, D = 4, 2048, 1024, 16, 64
HPC = 8          # heads per core
QC = 512         # q-chunk width
NT = T // 128    # 16 T-tiles of 128
NQC = T // QC    # 4 q-chunks
NKC = C // 128   # 8 contraction tiles for the input projections
NKP = (HPC * D) // 128  # 4 contraction tiles for the output projection
LAG = 3          # units between an S group and its AV group

# bisect flags (import os-env so variants don't need file edits)
import os
WV_ON_GPSIMD = os.environ.get("K_WV_GPSIMD", "0") == "1"
BCAST_GPSIMD = os.environ.get("K_BCAST_GPSIMD", "1") == "1"
EXP_FUSED = os.environ.get("K_EXP_FUSED", "1") == "1"

_CACHE = {}


def _build():
    nc = bacc.Bacc("TRN2", target_bir_lowering=False, debug=False)

    xT = nc.dram_tensor("xT", [C, T], F16, kind="ExternalInput")
    wqk = nc.dram_tensor("wqk", [C, HPC * 128], F16, kind="ExternalInput")
    bqk = nc.dram_tensor("bqk", [HPC * 128], F32, kind="ExternalInput")
    wv = nc.dram_tensor("wv", [C, HPC * D], F16, kind="ExternalInput")
    bv = nc.dram_tensor("bv", [HPC * D], F32, kind="ExternalInput")
    wp = nc.dram_tensor("wp", [HPC * D, C], F16, kind="ExternalInput")
    bp = nc.dram_tensor("bp", [C], F32, kind="ExternalInput")
    tri = nc.dram_tensor("tri", [128, 128], F16, kind="ExternalInput")
    outT = nc.dram_tensor("outT", [C, T], F32, kind="ExternalOutput")

    with tile.TileContext(nc) as tc:
        with (
            tc.tile_pool(name="wts", bufs=1) as wpool,
            tc.tile_pool(name="qk", bufs=1) as qkpool,
            tc.tile_pool(name="vy", bufs=1) as vypool,
            tc.tile_pool(name="xc", bufs=3) as xpool,
            tc.tile_pool(name="pt", bufs=5) as ptpool,
            tc.tile_pool(name="st", bufs=3) as stpool,
            tc.tile_pool(name="sm", bufs=3) as smallpool,
            tc.tile_pool(name="sg", bufs=2, space="PSUM") as spool,
            tc.tile_pool(name="ob", bufs=2, space="PSUM") as opool,
            tc.tile_pool(name="pp", bufs=2, space="PSUM") as ppool,
        ):
            # ---- persistent activation tiles ----
            q_t = [qkpool.tile([64, T], F16, tag=f"q{h}", name=f"q{h}") for h in range(HPC)]
            k_t = [qkpool.tile([64, T], F16, tag=f"k{h}", name=f"k{h}") for h in range(HPC)]
            v_sb = vypool.tile([128, NT, HPC, D + 1], F16, tag="v")
            nc.vector.memset(v_sb[:, :, :, D : D + 1], 1.0)
            y_t = [vypool.tile([128, T], F16, tag=f"y{g}", name=f"y{g}") for g in range(NKP)]

            # ---- startup DMAs: two big first loads on separate queues so
            # the chunk-0 V projection can start ~3us in; the rest streams
            # behind in consumption order ----
            xcs = {}
            def dma_xc(qc):
                xc = xpool.tile([128, NKC, QC], F16, tag="xc")
                src = xT.ap()[:, qc * QC : (qc + 1) * QC].rearrange("(a p) n -> p a n", p=128)
                if qc == 0:  # split so the first V m-tile is fed early
                    nc.sync.dma_start(xc[:, 0:4, :], src[:, 0:4, :])
                    nc.sync.dma_start(xc[:, 4:8, :], src[:, 4:8, :])
                else:
                    nc.sync.dma_start(xc[:], src)
                xcs[qc] = xc

            # one DMA ring (sync) in exact consumption order: first V m-tile
            # needs xc0 kk0-3 + wv kk0-3
            wv_sb = wpool.tile([128, NKC, HPC * D], F16, tag="wv")
            wv_src = wv.ap().rearrange("(a p) m -> p a m", p=128)
            xc0 = xpool.tile([128, NKC, QC], F16, tag="xc")
            xc0_src = xT.ap()[:, 0:QC].rearrange("(a p) n -> p a n", p=128)
            nc.sync.dma_start(xc0[:, 0:4, :], xc0_src[:, 0:4, :])
            nc.sync.dma_start(wv_sb[:, 0:4, :], wv_src[:, 0:4, :])
            nc.sync.dma_start(xc0[:, 4:8, :], xc0_src[:, 4:8, :])
            nc.sync.dma_start(wv_sb[:, 4:8, :], wv_src[:, 4:8, :])
            xcs[0] = xc0
            bvb = wpool.tile([128, HPC * D], F32, tag="bvb")
            nc.sync.dma_start(
                bvb[:],
                bv.ap().rearrange("(o n) -> o n", o=1).partition_broadcast(128),
            )
            bqk_sb = wpool.tile([128, HPC], F32, tag="bqk")
            nc.sync.dma_start(bqk_sb[:], bqk.ap().rearrange("(m p) -> p m", p=128))
            # two halves (m 0-3, 4-7): keeps 1KB contiguous runs (per-m slices
            # would cut runs to 256B and tank DMA efficiency) while letting
            # the first QK m-tiles start before the whole 2MB lands
            wqk_sb = wpool.tile([128, NKC, HPC * 128], F16, tag="wqk")
            wqk_src = wqk.ap().rearrange("(a p) m -> p a m", p=128)
            for half in range(2):
                s = slice(half * 512, (half + 1) * 512)
                nc.sync.dma_start(wqk_sb[:, :, s], wqk_src[:, :, s])
            dma_xc(1)
            trisb = wpool.tile([128, 128], F16, tag="tri")
            nc.sync.dma_start(trisb[:], tri.ap())
            wp_sb = wpool.tile([128, NKP, C], F16, tag="wp")
            nc.sync.dma_start(wp_sb[:], wp.ap().rearrange("(a p) m -> p a m", p=128))
            bp_sb = wpool.tile([128, C // 128], F32, tag="bp")
            nc.sync.dma_start(bp_sb[:], bp.ap().rearrange("(m p) -> p m", p=128))
            dma_xc(2)
            dma_xc(3)  # reuses xc slot 0; waits (briefly) on in-proj(0) reads

            # ---- projection m-tiles (also used as attention fillers) ----
            def vproj_mtile(qc, tt):
                ps = ppool.tile([128, QC], F32, tag="pp", name="psv")
                for kk in range(NKC):
                    nc.tensor.matmul(
                        ps[:],
                        xcs[qc][:, kk, tt * 128 : (tt + 1) * 128],
                        wv_sb[:, kk, :],
                        start=(kk == 0),
                        stop=(kk == NKC - 1),
                    )
                nc.vector.tensor_add(
                    v_sb[:, qc * 4 + tt, :, 0:D],
                    ps[:].rearrange("p (h d) -> p h d", d=D),
                    bvb[:].rearrange("p (h d) -> p h d", d=D),
                )

            def qkproj_mtile(qc, m):
                q0 = qc * QC
                ps = ppool.tile([128, QC], F32, tag="pp", name="psqk")
                for kk in range(NKC):
                    nc.tensor.matmul(
                        ps[:],
                        wqk_sb[:, kk, m * 128 : (m + 1) * 128],
                        xcs[qc][:, kk, :],
                        start=(kk == 0),
                        stop=(kk == NKC - 1),
                    )
                nc.vector.tensor_scalar_add(
                    q_t[m][:, q0 : q0 + QC], ps[0:64, :], bqk_sb[0:64, m : m + 1]
                )
                nc.vector.tensor_scalar_add(
                    k_t[m][:, q0 : q0 + QC], ps[64:128, :], bqk_sb[64:128, m : m + 1]
                )

            def oproj_mtile(qc, m):
                q0 = qc * QC
                pps = ppool.tile([128, QC], F32, tag="pp", name="pso")
                for kk in range(NKP):
                    nc.tensor.matmul(
                        pps[:],
                        wp_sb[:, kk, m * 128 : (m + 1) * 128],
                        y_t[kk][:, q0 : q0 + QC],
                        start=(kk == 0),
                        stop=(kk == NKP - 1),
                    )
                st = stpool.tile([128, QC], F32, tag="st", name="st")
                # alternate the bias-add/copy between ACT and DVE so neither
                # serializes the tail drain
                if m % 2 == 0:
                    nc.scalar.activation(st[:], pps[:], IDF, bias=bp_sb[:, m : m + 1])
                else:
                    nc.vector.tensor_scalar_add(st[:], pps[:], bp_sb[:, m : m + 1])
                nc.gpsimd.dma_start(
                    outT.ap()[m * 128 : (m + 1) * 128, q0 : q0 + QC], st[:]
                )

            def inproj_fillers(qc):
                return [lambda tt=tt: vproj_mtile(qc, tt) for tt in range(4)] + [
                    lambda m=m: qkproj_mtile(qc, m) for m in range(HPC)
                ]

            # ---- chunk 0 input projections up front ----
            for f in inproj_fillers(0):
                f()

            # ---- attention per chunk, fillers interleaved ----
            # unit = ("full", h, g) -> k-blocks 2g, 2g+1  (S pair + fused exp)
            #      | ("diag", h, o) -> k-block 4qc+o, partial width + tri mask
            def attention(qc, fillers):
                q0 = qc * QC
                n_k = 4 * qc + 4
                units = []
                for h in range(HPC):
                    for g in range(2 * qc):
                        units.append(("full", h, g))
                    for o in range(4):
                        units.append(("diag", h, o))
                nunits = len(units)
                uph = 2 * qc + 4  # units per head

                o_tiles = {}
                dsg = {}       # unit index -> sgrp tile for diag pairs
                pts = {}       # unit index -> pt tile

                def emit_S(i):
                    kind, h, a = units[i]
                    if kind == "full":
                        sg = spool.tile([128, 2, QC], F32, tag="sg", name="sg")
                        for b in (0, 1):
                            j = 2 * a + b
                            nc.tensor.matmul(
                                sg[:, b, :],
                                k_t[h][:, j * 128 : (j + 1) * 128],
                                q_t[h][:, q0 : q0 + QC],
                                start=True,
                                stop=True,
                            )
                        pt = ptpool.tile([128, 2, QC], F16, tag="pt2", name="pt2")
                        # per-bank exps: each bank's PSUM WAR clears as soon as
                        # its own exp is done, so the S pair reusing this tile
                        # two units later never waits on a 1us fused exp tail
                        for b in (0, 1):
                            nc.scalar.activation(
                                pt[:, b, :], sg[:, b, :], EXPF, scale=0.125
                            )
                    else:
                        o = a
                        if o % 2 == 0:
                            sg = spool.tile([128, 2, QC], F32, tag="sg", name="sgd")
                            dsg[i] = sg
                        else:
                            sg = dsg[i - 1]
                        j = 4 * qc + o
                        c0 = 128 * o
                        w = QC - c0
                        nc.tensor.matmul(
                            sg[:, o % 2, 0:w],
                            k_t[h][:, j * 128 : (j + 1) * 128],
                            q_t[h][:, q0 + c0 : q0 + QC],
                            start=True,
                            stop=True,
                        )
                        pt = ptpool.tile([128, QC], F16, tag="pt1", name="pt1")
                        nc.scalar.activation(
                            pt[:, 0:w], sg[:, o % 2, 0:w], EXPF, scale=0.125
                        )
                        nc.vector.tensor_mul(pt[:, 0:128], pt[:, 0:128], trisb[:])
                    pts[i] = pt

                def emit_AV(i):
                    kind, h, a = units[i]
                    if h not in o_tiles:
                        o_tiles[h] = opool.tile([128, QC], F32, tag="ob", name="ob")
                    ot = o_tiles[h]
                    pt = pts.pop(i)
                    if kind == "full":
                        for b in (0, 1):
                            j = 2 * a + b
                            nc.tensor.matmul(
                                ot[0:65, :],
                                v_sb[:, j, h, :],
                                pt[:, b, :],
                                start=(j == 0),
                                stop=(j == n_k - 1),
                            )
                    else:
                        o = a
                        j = 4 * qc + o
                        c0 = 128 * o
                        nc.tensor.matmul(
                            ot[0:65, c0:QC],
                            v_sb[:, j, h, :],
                            pt[:, 0 : QC - c0],
                            start=(j == 0),
                            stop=(j == n_k - 1),
                        )

                def emit_recip(h):
                    # copy l out of PSUM first: the approx-fast custom op's
                    # BITWISE_NOT seed reads garbage through the PSUM port
                    lrow = smallpool.tile([1, QC], F32, tag="lr", name="lr")
                    nc.vector.tensor_copy(lrow[:], o_tiles[h][64:65, :])
                    r = smallpool.tile([1, QC], F32, tag="r", name="r")
                    nc.vector.reciprocal_approx_fast(r[:], lrow[:])
                    return r

                def emit_norm(h, r):
                    rb = smallpool.tile([64, QC], F32, tag="rb", name="rb")
                    if BCAST_GPSIMD:
                        nc.gpsimd.partition_broadcast(rb[:], r[:])
                    else:
                        nc.sync.dma_start(rb[:], r[0:1, :].partition_broadcast(64))
                    r0 = (h % 2) * 64
                    nc.vector.tensor_mul(
                        y_t[h // 2][r0 : r0 + 64, q0 : q0 + QC],
                        o_tiles.pop(h)[0:64, :],
                        rb[:],
                    )

                recips = {}
                fi = 0
                nf = len(fillers)
                for i in range(nunits + LAG + 2):
                    if i < nunits:
                        emit_S(i)
                    iav = i - LAG
                    if 0 <= iav < nunits:
                        emit_AV(iav)
                        if (iav + 1) % uph == 0:  # last unit of its head
                            h = units[iav][1]
                            recips[h] = emit_recip(h)
                    inorm = i - LAG - 2
                    if 0 <= inorm < nunits and (inorm + 1) % uph == 0:
                        h = units[inorm][1]
                        emit_norm(h, recips.pop(h))
                    # keep spreading fillers through the drain steps too, so
                    # the PE has cover while the last exps/flushes resolve
                    while fi < nf and fi * (nunits + LAG + 2) < (i + 1) * nf:
                        fillers[fi]()
                        fi += 1
                while fi < nf:
                    fillers[fi]()
                    fi += 1

            attention(0, inproj_fillers(1))
            attention(1, inproj_fillers(2))
            attention(2, inproj_fillers(3))
            attention(
                3,
                [lambda qc=qc, m=m: oproj_mtile(qc, m) for qc in range(3) for m in range(HPC)],
            )
            for m in range(HPC):
                oproj_mtile(3, m)

    nc.compile()
    return nc


def _shards(W_attn, b_attn, W_proj, b_proj):
    """Per-TP-half weight shards (t = 0, 1), packed for the device layout."""
    shards = []
    for t in range(2):
        heads = range(t * HPC, (t + 1) * HPC)
        wqk = np.empty((C, HPC * 128), np.float16)
        bqk = np.empty(HPC * 128, np.float32)
        wv = np.empty((C, HPC * D), np.float16)
        bvv = np.empty(HPC * D, np.float32)
        for j, h in enumerate(heads):
            wqk[:, j * 128 : j * 128 + 64] = W_attn[:, h * D : (h + 1) * D]
            wqk[:, j * 128 + 64 : j * 128 + 128] = W_attn[:, C + h * D : C + (h + 1) * D]
            bqk[j * 128 : j * 128 + 64] = b_attn[h * D : (h + 1) * D]
            bqk[j * 128 + 64 : j * 128 + 128] = b_attn[C + h * D : C + (h + 1) * D]
            wv[:, j * D : (j + 1) * D] = W_attn[:, 2 * C + h * D : 2 * C + (h + 1) * D]
            bvv[j * D : (j + 1) * D] = b_attn[2 * C + h * D : 2 * C + (h + 1) * D]
        wp = W_proj[t * HPC * D : (t + 1) * HPC * D, :].astype(np.float16)
        bpp = (b_proj if t == 0 else np.zeros_like(b_proj)).astype(np.float32)
        shards.append((wqk, bqk, wv, bvv, np.ascontiguousarray(wp), bpp))
    return shards


def _tri_np():
    kr = np.arange(128)[:, None]
    cc = np.arange(128)[None, :]
    return (kr <= cc).astype(np.float16)


def _in_maps(x, W_attn, b_attn, W_proj, b_proj):
    shards = _shards(W_attn, b_attn, W_proj, b_proj)
    tri = _tri_np()
    in_maps = []
    for b in range(B):
        xTb = np.ascontiguousarray(x[b].T.astype(np.float16))
        for t in range(2):
            wqk, bqk, wv, bvv, wp, bpp = shards[t]
            in_maps.append(
                {
                    "xT": xTb,
                    "wqk": wqk,
                    "bqk": bqk,
                    "wv": wv,
                    "bv": bvv,
                    "wp": wp,
                    "bp": bpp,
                    "tri": tri,
                }
            )
    return in_maps


def _gather(results):
    out = np.empty((B, T, C), np.float32)
    for b in range(B):
        acc = results[2 * b]["outT"] + results[2 * b + 1]["outT"]
        out[b] = acc.T
    return out


def kernel(x, W_attn, b_attn, W_proj, b_proj):
    x = np.asarray(x, np.float32)
    W_attn = np.asarray(W_attn, np.float32)
    b_attn = np.asarray(b_attn, np.float32)
    W_proj = np.asarray(W_proj, np.float32)
    b_proj = np.asarray(b_proj, np.float32)

    if "nc" not in _CACHE:
        _CACHE["nc"] = _build()
    nc = _CACHE["nc"]

    in_maps = _in_maps(x, W_attn, b_attn, W_proj, b_proj)
    res = run_bass_kernel_spmd(nc, in_maps, core_ids=list(range(8)))
    return _gather(res.results)


# revision 25
# speedup vs baseline: 1.4315x; 1.0267x over previous
"""Causal self-attention (B=4, T=2048, C=1024, H=16) on 8 Trainium2 cores.

Sharding: data-parallel over batch (4 groups) x tensor-parallel over heads
(2-way). Core c = 2*b + t handles batch b, heads [t*8, t*8+8).

v2 design notes (PE-saturation schedule):
  The PE clock is governed by a DVFS controller: sustained ~100% utilization
  holds ~2.37 GHz; utilization gaps in its ~3.4us evaluation window drop it
  to ~1.2 GHz with a ~3us ramp back. The v1 kernel lost ~45-65% of its span
  to that downshift because the S->exp->AV chain stalled the PE behind the
  scalar engine. v2 restructures so the PE queue never waits:

  - attention emitted in units (pairs of 128-row k-blocks; diagonal blocks
    single) with a 2-unit lag between each unit's S matmuls and its AV
    matmuls, so every exp has ~2us of PE cover.
  - exps fused across 2 PSUM banks ([128,1024] per ACT op) to amortize the
    ~190ns ACT fixed overhead; diagonal blocks keep partial widths.
  - causal masking is multiplicative AFTER exp (f16 DVE mul against a
    [128,128] triangle tile, only on the mixed 128-col region) instead of
    additive -1e30 before it: halves DVE cost and keeps exp input PSUM-fused.
  - input projections of chunk qc+1 and the output projections are emitted
    as m-tile fillers INSIDE attention so proj work absorbs any PE slack.
  - softmax normalize without DRAM round-trips: DVE reciprocal of the l row
    (PSUM partition 64), gpsimd partition_broadcast, then one DVE mul from
    PSUM O straight into y^T f16.
  - V bias via DVE add against a broadcast tile (frees the PE ones-matmul).

Per-core math (all matmuls fp16 in / fp32 psum accumulate), as in v1:
  qkv in transposed layout q^T/k^T[feat,T]; V in [T,feat] with a ones column
  so AV also accumulates the softmax denominator; S^T = K^T.T @ Q^T per
  (head, k-block, 512-col q-chunk), causal blocks only; out^T row-parallel =
  W_proj_half.T @ y^T (+ b_proj on the t=0 core); host sums TP partials.
"""

import sys

import numpy as np

from concourse import mybir, tile, bacc
from concourse import bass_utils
from concourse.bass_utils import run_bass_kernel_spmd


def _ensure_trace_support():
    """Make trace=True / BASS_TRACE runs survive on images whose antenv lacks
    axon_hooks and where artifact upload has no credentials. Both shims are
    no-ops on the untraced path."""
    try:
        import antenv.axon_hooks  # noqa: F401
    except ImportError:
        import contextlib
        import ctypes
        import os
        import types

        mod = types.ModuleType("antenv.axon_hooks")
        state = {"hook": None, "tried": False}

        def set_axon_ntff_profile_hook(hook):
            state["hook"] = hook

        def _via_ctypes(so_path):
            lib = ctypes.CDLL(so_path)
            if not hasattr(lib, "axon_start_nrt_profile"):
                return None
            lib.axon_start_nrt_profile.argtypes = [
                ctypes.POINTER(ctypes.c_int64),
                ctypes.c_size_t,
            ]
            lib.axon_start_nrt_profile.restype = ctypes.c_int64
            lib.axon_stop_nrt_profile.argtypes = [ctypes.c_char_p]
            lib.axon_stop_nrt_profile.restype = ctypes.c_int64

            @contextlib.contextmanager
            def _hook(output_dir, device_ids):
                import jax

                jax.devices()
                if device_ids:
                    ids = (ctypes.c_int64 * len(device_ids))(*device_ids)
                    rc = lib.axon_start_nrt_profile(ids, len(device_ids))
                else:
                    rc = lib.axon_start_nrt_profile(None, 0)
                if rc != 0:
                    raise RuntimeError(f"axon_start_nrt_profile rc={rc}")
                try:
                    yield
                finally:
                    lib.axon_stop_nrt_profile(str(output_dir).encode())

            return _hook

        def get_axon_ntff_profile_hook():
            if state["hook"] is None and not state["tried"]:
                state["tried"] = True
                so = os.environ.get("AXON_PJRT_SO", "/opt/axon/libaxon_pjrt.so")
                if os.path.exists(so):
                    try:
                        state["hook"] = _via_ctypes(so)
                    except OSError:
                        pass
            return state["hook"]

        mod.set_axon_ntff_profile_hook = set_axon_ntff_profile_hook
        mod.get_axon_ntff_profile_hook = get_axon_ntff_profile_hook
        sys.modules["antenv.axon_hooks"] = mod

    orig_upload = bass_utils.upload_artifacts
    if not getattr(orig_upload, "_safe_wrapped", False):
        def _safe_upload(tmpdir):
            try:
                return orig_upload(tmpdir)
            except Exception:
                return "local://" + str(tmpdir)

        _safe_upload._safe_wrapped = True
        bass_utils.upload_artifacts = _safe_upload


_ensure_trace_support()

F16 = mybir.dt.float16
F32 = mybir.dt.float32
EXPF = mybir.ActivationFunctionType.Exp
IDF = mybir.ActivationFunctionType.Identity

B, T, C, H, D = 4, 2048, 1024, 16, 64
HPC = 8          # heads per core
QC = 512         # q-chunk width
NT = T // 128    # 16 T-tiles of 128
NQC = T // QC    # 4 q-chunks
NKC = C // 128   # 8 contraction tiles for the input projections
NKP = (HPC * D) // 128  # 4 contraction tiles for the output projection
LAG = 3          # units between an S group and its AV group

# bisect flags (import os-env so variants don't need file edits)
import os
WV_ON_GPSIMD = os.environ.get("K_WV_GPSIMD", "0") == "1"
BCAST_GPSIMD = os.environ.get("K_BCAST_GPSIMD", "1") == "1"
EXP_FUSED = os.environ.get("K_EXP_FUSED", "1") == "1"

_CACHE = {}


def _build():
    nc = bacc.Bacc("TRN2", target_bir_lowering=False, debug=False)

    xT = nc.dram_tensor("xT", [C, T], F16, kind="ExternalInput")
    wqk = nc.dram_tensor("wqk", [C, HPC * 128], F16, kind="ExternalInput")
    bqk = nc.dram_tensor("bqk", [HPC * 128], F32, kind="ExternalInput")
    wv = nc.dram_tensor("wv", [C, HPC * D], F16, kind="ExternalInput")
    bv = nc.dram_tensor("bv", [HPC * D], F32, kind="ExternalInput")
    wp = nc.dram_tensor("wp", [HPC * D, C], F16, kind="ExternalInput")
    bp = nc.dram_tensor("bp", [C], F32, kind="ExternalInput")
    tri = nc.dram_tensor("tri", [128, 128], F16, kind="ExternalInput")
    outT = nc.dram_tensor("outT", [C, T], F32, kind="ExternalOutput")

    with tile.TileContext(nc) as tc:
        with (
            tc.tile_pool(name="wts", bufs=1) as wpool,
            tc.tile_pool(name="qk", bufs=1) as qkpool,
            tc.tile_pool(name="vy", bufs=1) as vypool,
            tc.tile_pool(name="xc", bufs=3) as xpool,
            tc.tile_pool(name="pt", bufs=5) as ptpool,
            tc.tile_pool(name="st", bufs=3) as stpool,
            tc.tile_pool(name="sm", bufs=3) as smallpool,
            tc.tile_pool(name="sg", bufs=2, space="PSUM") as spool,
            tc.tile_pool(name="ob", bufs=2, space="PSUM") as opool,
            tc.tile_pool(name="pp", bufs=2, space="PSUM") as ppool,
        ):
            # ---- persistent activation tiles ----
            q_t = [qkpool.tile([64, T], F16, tag=f"q{h}", name=f"q{h}") for h in range(HPC)]
            k_t = [qkpool.tile([64, T], F16, tag=f"k{h}", name=f"k{h}") for h in range(HPC)]
            v_sb = vypool.tile([128, NT, HPC, D + 1], F16, tag="v")
            nc.vector.memset(v_sb[:, :, :, D : D + 1], 1.0)
            y_t = [vypool.tile([128, T], F16, tag=f"y{g}", name=f"y{g}") for g in range(NKP)]

            # ---- startup DMAs: two big first loads on separate queues so
            # the chunk-0 V projection can start ~3us in; the rest streams
            # behind in consumption order ----
            xcs = {}
            def dma_xc(qc):
                xc = xpool.tile([128, NKC, QC], F16, tag="xc")
                src = xT.ap()[:, qc * QC : (qc + 1) * QC].rearrange("(a p) n -> p a n", p=128)
                if qc == 0:  # split so the first V m-tile is fed early
                    nc.sync.dma_start(xc[:, 0:4, :], src[:, 0:4, :])
                    nc.sync.dma_start(xc[:, 4:8, :], src[:, 4:8, :])
                else:
                    nc.sync.dma_start(xc[:], src)
                xcs[qc] = xc

            # one DMA ring (sync) in exact consumption order: first V m-tile
            # needs xc0 kk0-3 + wv kk0-3
            wv_sb = wpool.tile([128, NKC, HPC * D], F16, tag="wv")
            wv_src = wv.ap().rearrange("(a p) m -> p a m", p=128)
            xc0 = xpool.tile([128, NKC, QC], F16, tag="xc")
            xc0_src = xT.ap()[:, 0:QC].rearrange("(a p) n -> p a n", p=128)
            nc.sync.dma_start(xc0[:, 0:4, :], xc0_src[:, 0:4, :])
            nc.sync.dma_start(wv_sb[:, 0:4, :], wv_src[:, 0:4, :])
            nc.sync.dma_start(xc0[:, 4:8, :], xc0_src[:, 4:8, :])
            nc.sync.dma_start(wv_sb[:, 4:8, :], wv_src[:, 4:8, :])
            xcs[0] = xc0
            bvb = wpool.tile([128, HPC * D], F32, tag="bvb")
            nc.sync.dma_start(
                bvb[:],
                bv.ap().rearrange("(o n) -> o n", o=1).partition_broadcast(128),
            )
            bqk_sb = wpool.tile([128, HPC], F32, tag="bqk")
            nc.sync.dma_start(bqk_sb[:], bqk.ap().rearrange("(m p) -> p m", p=128))
            # two halves (m 0-3, 4-7): keeps 1KB contiguous runs (per-m slices
            # would cut runs to 256B and tank DMA efficiency) while letting
            # the first QK m-tiles start before the whole 2MB lands
            wqk_sb = wpool.tile([128, NKC, HPC * 128], F16, tag="wqk")
            wqk_src = wqk.ap().rearrange("(a p) m -> p a m", p=128)
            for half in range(2):
                s = slice(half * 512, (half + 1) * 512)
                nc.sync.dma_start(wqk_sb[:, :, s], wqk_src[:, :, s])
            dma_xc(1)
            trisb = wpool.tile([128, 128], F16, tag="tri")
            nc.sync.dma_start(trisb[:], tri.ap())
            wp_sb = wpool.tile([128, NKP, C], F16, tag="wp")
            nc.sync.dma_start(wp_sb[:], wp.ap().rearrange("(a p) m -> p a m", p=128))
            bp_sb = wpool.tile([128, C // 128], F32, tag="bp")
            nc.sync.dma_start(bp_sb[:], bp.ap().rearrange("(m p) -> p m", p=128))
            dma_xc(2)
            dma_xc(3)  # reuses xc slot 0; waits (briefly) on in-proj(0) reads

            # ---- projection m-tiles (also used as attention fillers) ----
            def vproj_mtile(qc, tt):
                ps = ppool.tile([128, QC], F32, tag="pp", name="psv")
                for kk in range(NKC):
                    nc.tensor.matmul(
                        ps[:],
                        xcs[qc][:, kk, tt * 128 : (tt + 1) * 128],
                        wv_sb[:, kk, :],
                        start=(kk == 0),
                        stop=(kk == NKC - 1),
                    )
                nc.vector.tensor_add(
                    v_sb[:, qc * 4 + tt, :, 0:D],
                    ps[:].rearrange("p (h d) -> p h d", d=D),
                    bvb[:].rearrange("p (h d) -> p h d", d=D),
                )

            def qkproj_mtile(qc, m):
                q0 = qc * QC
                ps = ppool.tile([128, QC], F32, tag="pp", name="psqk")
                for kk in range(NKC):
                    nc.tensor.matmul(
                        ps[:],
                        wqk_sb[:, kk, m * 128 : (m + 1) * 128],
                        xcs[qc][:, kk, :],
                        start=(kk == 0),
                        stop=(kk == NKC - 1),
                    )
                nc.vector.tensor_scalar_add(
                    q_t[m][:, q0 : q0 + QC], ps[0:64, :], bqk_sb[0:64, m : m + 1]
                )
                nc.vector.tensor_scalar_add(
                    k_t[m][:, q0 : q0 + QC], ps[64:128, :], bqk_sb[64:128, m : m + 1]
                )

            def oproj_mtile(qc, m):
                q0 = qc * QC
                pps = ppool.tile([128, QC], F32, tag="pp", name="pso")
                for kk in range(NKP):
                    nc.tensor.matmul(
                        pps[:],
                        wp_sb[:, kk, m * 128 : (m + 1) * 128],
                        y_t[kk][:, q0 : q0 + QC],
                        start=(kk == 0),
                        stop=(kk == NKP - 1),
                    )
                st = stpool.tile([128, QC], F32, tag="st", name="st")
                # alternate the bias-add/copy between ACT and DVE so neither
                # serializes the tail drain
                if m % 2 == 0:
                    nc.scalar.activation(st[:], pps[:], IDF, bias=bp_sb[:, m : m + 1])
                else:
                    nc.vector.tensor_scalar_add(st[:], pps[:], bp_sb[:, m : m + 1])
                nc.gpsimd.dma_start(
                    outT.ap()[m * 128 : (m + 1) * 128, q0 : q0 + QC], st[:]
                )

            def inproj_fillers(qc):
                return [lambda tt=tt: vproj_mtile(qc, tt) for tt in range(4)] + [
                    lambda m=m: qkproj_mtile(qc, m) for m in range(HPC)
                ]

            # ---- chunk 0 input projections up front ----
            for f in inproj_fillers(0):
                f()

            # ---- attention: one global unit stream across all chunks ----
            # unit = ("full", qc, h, g) -> k-blocks 2g, 2g+1 (S pair + fused exp)
            #      | ("diag", qc, h, o) -> k-block 4qc+o, partial width + tri mask
            # Chunk 3's non-diagonal pairs only need K/V from chunks <= 2, so
            # they are scheduled in their own phase between chunk 2 and chunk
            # 3's diagonal, overlapping the output projections; this keeps the
            # scalar engine's exp load spread evenly under the PE's work.
            phases = []
            phases.append((
                [("diag", 0, h, o) for h in range(HPC) for o in range(4)],
                inproj_fillers(1),
            ))
            phases.append((
                [(k, 1, h, a) for h in range(HPC)
                 for (k, a) in [("full", 0), ("full", 1)] + [("diag", o) for o in range(4)]],
                inproj_fillers(2),
            ))
            phases.append((
                [(k, 2, h, a) for h in range(HPC)
                 for (k, a) in [("full", g) for g in range(4)] + [("diag", o) for o in range(4)]],
                inproj_fillers(3) + [lambda m=m: oproj_mtile(0, m) for m in range(HPC)],
            ))
            phases.append((
                [(k, 3, h, a) for h in range(HPC)
                 for (k, a) in [("full", g) for g in range(6)] + [("diag", o) for o in range(4)]],
                [lambda m=m: oproj_mtile(1, m) for m in range(HPC)]
                + [lambda m=m: oproj_mtile(2, m) for m in range(HPC)],
            ))

            units = []
            fill_at = {}
            for us, fills in phases:
                s, e = len(units), len(units) + len(us)
                units.extend(us)
                span = e - s if us else 1
                for idx, f in enumerate(fills):
                    pos = s + (idx + 1) * span // (len(fills) + 1)
                    fill_at.setdefault(pos, []).append(f)
            nunits = len(units)

            o_tiles = {}   # (qc, h) -> O psum tile
            dsg = {}       # (qc, h) -> sgrp tile shared by diag pairs
            pts = {}       # unit index -> pt tile

            def emit_S(i):
                kind, qc, h, a = units[i]
                q0 = qc * QC
                if kind == "full":
                    sg = spool.tile([128, 2, QC], F32, tag="sg", name="sg")
                    for b in (0, 1):
                        j = 2 * a + b
                        nc.tensor.matmul(
                            sg[:, b, :],
                            k_t[h][:, j * 128 : (j + 1) * 128],
                            q_t[h][:, q0 : q0 + QC],
                            start=True,
                            stop=True,
                        )
                    pt = ptpool.tile([128, 2, QC], F16, tag="pt2", name="pt2")
                    nc.scalar.activation(
                        pt[:].rearrange("p a n -> p (a n)"),
                        sg[:].rearrange("p a n -> p (a n)"),
                        EXPF,
                        scale=0.125,
                    )
                else:
                    o = a
                    if o % 2 == 0:
                        sg = spool.tile([128, 2, QC], F32, tag="sg", name="sgd")
                        dsg[(qc, h)] = sg
                    else:
                        sg = dsg[(qc, h)]
                    j = 4 * qc + o
                    c0 = 128 * o
                    w = QC - c0
                    nc.tensor.matmul(
                        sg[:, o % 2, 0:w],
                        k_t[h][:, j * 128 : (j + 1) * 128],
                        q_t[h][:, q0 + c0 : q0 + QC],
                        start=True,
                        stop=True,
                    )
                    pt = ptpool.tile([128, QC], F16, tag="pt1", name="pt1")
                    nc.scalar.activation(
                        pt[:, 0:w], sg[:, o % 2, 0:w], EXPF, scale=0.125
                    )
                    nc.vector.tensor_mul(pt[:, 0:128], pt[:, 0:128], trisb[:])
                pts[i] = pt

            def emit_AV(i):
                kind, qc, h, a = units[i]
                n_k = 4 * qc + 4
                if (qc, h) not in o_tiles:
                    o_tiles[(qc, h)] = opool.tile([128, QC], F32, tag="ob", name="ob")
                ot = o_tiles[(qc, h)]
                pt = pts.pop(i)
                if kind == "full":
                    for b in (0, 1):
                        j = 2 * a + b
                        nc.tensor.matmul(
                            ot[0:65, :],
                            v_sb[:, j, h, :],
                            pt[:, b, :],
                            start=(j == 0),
                            stop=(j == n_k - 1),
                        )
                else:
                    o = a
                    j = 4 * qc + o
                    c0 = 128 * o
                    nc.tensor.matmul(
                        ot[0:65, c0:QC],
                        v_sb[:, j, h, :],
                        pt[:, 0 : QC - c0],
                        start=(j == 0),
                        stop=(j == n_k - 1),
                    )

            def emit_recip(qc, h):
                # copy l out of PSUM first: the approx-fast custom op's
                # BITWISE_NOT seed reads garbage through the PSUM port
                lrow = smallpool.tile([1, QC], F32, tag="lr", name="lr")
                nc.vector.tensor_copy(lrow[:], o_tiles[(qc, h)][64:65, :])
                r = smallpool.tile([1, QC], F32, tag="r", name="r")
                nc.vector.reciprocal_approx_fast(r[:], lrow[:])
                return r

            def emit_norm(qc, h, r):
                rb = smallpool.tile([64, QC], F32, tag="rb", name="rb")
                nc.gpsimd.partition_broadcast(rb[:], r[:])
                r0 = (h % 2) * 64
                nc.vector.tensor_mul(
                    y_t[h // 2][r0 : r0 + 64, qc * QC : (qc + 1) * QC],
                    o_tiles.pop((qc, h))[0:64, :],
                    rb[:],
                )

            recips = {}
            for i in range(nunits + LAG + 2):
                if i < nunits:
                    emit_S(i)
                iav = i - LAG
                if 0 <= iav < nunits:
                    u = units[iav]
                    emit_AV(iav)
                    if u[0] == "diag" and u[3] == 3:  # head-chunk complete
                        recips[(u[1], u[2])] = emit_recip(u[1], u[2])
                inorm = i - LAG - 2
                if 0 <= inorm < nunits:
                    u = units[inorm]
                    if u[0] == "diag" and u[3] == 3:
                        emit_norm(u[1], u[2], recips.pop((u[1], u[2])))
                for f in fill_at.get(i, ()):
                    f()
            for m in range(HPC):
                oproj_mtile(3, m)

    nc.compile()
    return nc


def _shards(W_attn, b_attn, W_proj, b_proj):
    """Per-TP-half weight shards (t = 0, 1), packed for the device layout."""
    shards = []
    for t in range(2):
        heads = range(t * HPC, (t + 1) * HPC)
        wqk = np.empty((C, HPC * 128), np.float16)
        bqk = np.empty(HPC * 128, np.float32)
        wv = np.empty((C, HPC * D), np.float16)
        bvv = np.empty(HPC * D, np.float32)
        for j, h in enumerate(heads):
            wqk[:, j * 128 : j * 128 + 64] = W_attn[:, h * D : (h + 1) * D]
            wqk[:, j * 128 + 64 : j * 128 + 128] = W_attn[:, C + h * D : C + (h + 1) * D]
            bqk[j * 128 : j * 128 + 64] = b_attn[h * D : (h + 1) * D]
            bqk[j * 128 + 64 : j * 128 + 128] = b_attn[C + h * D : C + (h + 1) * D]
            wv[:, j * D : (j + 1) * D] = W_attn[:, 2 * C + h * D : 2 * C + (h + 1) * D]
            bvv[j * D : (j + 1) * D] = b_attn[2 * C + h * D : 2 * C + (h + 1) * D]
        wp = W_proj[t * HPC * D : (t + 1) * HPC * D, :].astype(np.float16)
        bpp = (b_proj if t == 0 else np.zeros_like(b_proj)).astype(np.float32)
        shards.append((wqk, bqk, wv, bvv, np.ascontiguousarray(wp), bpp))
    return shards


def _tri_np():
    kr = np.arange(128)[:, None]
    cc = np.arange(128)[None, :]
    return (kr <= cc).astype(np.float16)


def _in_maps(x, W_attn, b_attn, W_proj, b_proj):
    shards = _shards(W_attn, b_attn, W_proj, b_proj)
    tri = _tri_np()
    in_maps = []
    for b in range(B):
        xTb = np.ascontiguousarray(x[b].T.astype(np.float16))
        for t in range(2):
            wqk, bqk, wv, bvv, wp, bpp = shards[t]
            in_maps.append(
                {
                    "xT": xTb,
                    "wqk": wqk,
                    "bqk": bqk,
                    "wv": wv,
                    "bv": bvv,
                    "wp": wp,
                    "bp": bpp,
                    "tri": tri,
                }
            )
    return in_maps


def _gather(results):
    out = np.empty((B, T, C), np.float32)
    for b in range(B):
        acc = results[2 * b]["outT"] + results[2 * b + 1]["outT"]
        out[b] = acc.T
    return out


def kernel(x, W_attn, b_attn, W_proj, b_proj):
    x = np.asarray(x, np.float32)
    W_attn = np.asarray(W_attn, np.float32)
    b_attn = np.asarray(b_attn, np.float32)
    W_proj = np.asarray(W_proj, np.float32)
    b_proj = np.asarray(b_proj, np.float32)

    if "nc" not in _CACHE:
        _CACHE["nc"] = _build()
    nc = _CACHE["nc"]

    in_maps = _in_maps(x, W_attn, b_attn, W_proj, b_proj)
    res = run_bass_kernel_spmd(nc, in_maps, core_ids=list(range(8)))
    return _gather(res.results)
